# revision 13
# baseline (speedup 1.0000x reference)
"""KNN-conv kernel for Trainium2, data-parallel over batch on 8 NeuronCores.

Problem: for x (32, 128, 32, 32) and conv weight W (128, 128, 9):
  per batch: cosine-sim (1024x1024) over channels, diag -> -1e10, top-8
  neighbors per token (+ self as rank 0), gather neighbor features,
  contract with W.

Strategy per core (4 batches), software-pipelined at sim-block
granularity: batch b's similarity/top-k blocks carry interleaved work
from neighboring batches (conv halves of b-2, normalization of b+1) so
every engine queue stays busy and the PE stays ramped:
  A: load x, xn = x * 1/sqrt(sum x^2) (Act square/sqrt, DVE recip, Pool
     partition_broadcast + multiply, norms bounced through DRAM to get
     token-major layout)
  B: sim in true-fp32 PE matmuls (top-k selection needs ~17 mantissa
     bits; bf16/tf32-class dtypes flip ~1-3% of picks = rel err >> 2e-2),
     diag forced to -1e10 by an accumulating identity matmul, top-8 via
     DVE max8 + max_index per 128-row block
  C: index shuffle into the dma_gather wrap layout. Gather position m
     holds token w(m), where w swaps the block/slot bit-fields
     (m = 128e+16c+q -> t = 128c+16e+q), which makes the shuffle DMAs
     16B-run / contiguous instead of 2B-element scatters. Two
     dma_gather(transpose=True) calls fetch 4 neighbor ranks each from
     token-major bf16 rows (256B each).
  D: conv = single-pass bf16 matmuls (hi-only features, ~0.3% extra
     error, no top-k amplification) accumulated in fp32 PSUM; output
     columns are in wrap order, host unpermutes (w is an involution).
"""

import os

import numpy as np

B, C, N, K = 32, 128, 1024, 9
O = 128  # out channels
NCORES = 8
BPC = B // NCORES  # batches per core
NEG = -1.0e10

_prog_cache = {}
last_results = None  # BassKernelResults of the most recent run (for test.py)


def _wrap_perm():
    """w(m): swap the high (e) and mid (c) 3-bit fields of m = 128e+16c+q."""
    m = np.arange(N)
    e, c, q = m >> 7, (m >> 4) & 7, m & 15
    return (c << 7) | (e << 4) | q


def _build_program():
    import concourse.bacc as bacc
    import concourse.mybir as mybir
    from concourse.tile import TileContext

    f32 = mybir.dt.float32
    bf16 = mybir.dt.bfloat16
    u16 = mybir.dt.uint16
    i16 = mybir.dt.int16
    AF = mybir.ActivationFunctionType

    nc = bacc.Bacc()
    skip = set(os.environ.get("KNN_SKIP", "").split(","))
    hilo = os.environ.get("KNN_SIM", "f32") == "hilo"

    x_h = nc.declare_dram_parameter("x", [BPC, C, N], f32, isOutput=False)
    xt_h = nc.declare_dram_parameter("xt", [BPC, N, C], bf16, isOutput=False)
    hiw_h = nc.declare_dram_parameter("hiw", [BPC, C, N], bf16, isOutput=False)
    wt_h = nc.declare_dram_parameter("wt", [C, K * O], bf16, isOutput=False)
    ident_h = nc.declare_dram_parameter("ident", [128, 128], bf16, isOutput=False)
    negi_h = nc.declare_dram_parameter("negi", [128, 128], bf16, isOutput=False)
    ones128_h = nc.declare_dram_parameter("ones128", [C, 1], f32, isOutput=False)
    out_h = nc.declare_dram_parameter("out", [BPC, O, N], f32, isOutput=True)

    idxd_h = nc.dram_tensor("idxd", [BPC, 8192], u16)
    rd_h = nc.dram_tensor("rd", [BPC, N], f32)

    with TileContext(nc) as tc:
        with (
            tc.tile_pool(name="consts", bufs=1) as consts,
            tc.tile_pool(name="xp", bufs=3) as xp,
            tc.tile_pool(name="sqp", bufs=2) as sqp,
            tc.tile_pool(name="xnp", bufs=2) as xnp,
            tc.tile_pool(name="xnhp", bufs=2) as xnhp,
            tc.tile_pool(name="xnlp", bufs=2) as xnlp,
            tc.tile_pool(name="rp", bufs=2) as rp,
            tc.tile_pool(name="normp", bufs=6) as normp,
            tc.tile_pool(name="scp", bufs=int(os.environ.get("KNN_SCP", "3"))) as scp,
            tc.tile_pool(name="v8p", bufs=3) as v8p,
            tc.tile_pool(name="idxp", bufs=2) as idxp,
            tc.tile_pool(name="idxgp", bufs=2) as idxgp,
            tc.tile_pool(name="prp", bufs=int(os.environ.get("KNN_PRP", "6"))) as prp,
            tc.tile_pool(name="hlp", bufs=3) as hlp,
            tc.tile_pool(name="outp", bufs=2) as outp,
            tc.tile_pool(name="psb", bufs=int(os.environ.get("KNN_PSB", "6")), space="PSUM") as psb,
            tc.tile_pool(name="pso", bufs=2, space="PSUM") as pso,
        ):
            wts = consts.tile([C, K * O], bf16, tag="wts")
            nc.sync.dma_start(out=wts[:], in_=wt_h[:])
            ident = consts.tile([128, 128], bf16, tag="ident")
            nc.sync.dma_start(out=ident[:], in_=ident_h[:])
            negi = consts.tile([128, 128], bf16, tag="negi")
            nc.sync.dma_start(out=negi[:], in_=negi_h[:])
            ones128 = consts.tile([C, 1], f32, tag="ones128")
            nc.sync.dma_start(out=ones128[:], in_=ones128_h[:])

            st = {}  # per-batch live tiles

            def emit_A1(b):
                # x load + square
                X = xp.tile([C, N], f32, tag="x")
                nc.sync.dma_start(out=X[:], in_=x_h[b])
                SQ = sqp.tile([C, N], f32, tag="sq")
                nc.scalar.activation(SQ[:], X[:], AF.Square)
                st[b] = {"X": X, "SQ": SQ}

            def emit_A2a(b):
                # norm^2 transposed: n2[p, blk] = sum_c SQ[c, blk*128+p]
                SQ = st[b]["SQ"]
                n2 = psb.tile([128, 512], f32, tag="ps_big")
                for blk in range(8):
                    nc.tensor.matmul(
                        n2[:, blk : blk + 1],
                        SQ[:, blk * 128 : (blk + 1) * 128],
                        ones128[:],
                        start=True,
                        stop=True,
                    )
                sq8 = normp.tile([128, 8], f32, tag="sq8")
                nc.scalar.activation(sq8[:], n2[:, :8], AF.Sqrt)
                st[b]["sq8"] = sq8

            def emit_A2b(b):
                # reciprocal + token-major bounce + broadcast + multiply
                X, sq8 = st[b]["X"], st[b]["sq8"]
                rT = normp.tile([128, 8], f32, tag="rT")
                nc.vector.reciprocal(rT[:], sq8[:])
                nc.sync.dma_start(
                    out=rd_h[b].rearrange("(blk p) -> p blk", p=128), in_=rT[:]
                )
                r1 = normp.tile([1, N], f32, tag="r1")
                nc.sync.dma_start(
                    out=r1[:], in_=rd_h[b].rearrange("(one n) -> one n", one=1)
                )
                R = rp.tile([128, N], f32, tag="r")
                nc.gpsimd.partition_broadcast(R[:], r1[:])
                XN = xnp.tile([C, N], f32, tag="xn")
                nc.gpsimd.tensor_mul(XN[:], X[:], R[:])
                st[b]["XN"] = XN
                if hilo:
                    XNh = xnhp.tile([C, N], bf16, tag="xnh")
                    nc.scalar.activation(XNh[:], XN[:], AF.Copy)
                    XNl = xnlp.tile([C, N], bf16, tag="xnl")
                    nc.gpsimd.tensor_sub(XNl[:], XN[:], XNh[:])
                    st[b]["XNh"], st[b]["XNl"] = XNh, XNl

            def emit_sim_block(b, c):
                # one 128-row sim block + top-8 scan
                if "sim" in skip:
                    nc.vector.memset(st[b]["IDX"][:, c : 64 : 8], c)
                    return
                sl = slice(c * 128, (c + 1) * 128)
                SC = scp.tile([128, N], f32, tag="sc")
                for h in range(2):
                    ps = psb.tile([128, 512], f32, tag="ps_big")
                    fs = slice(h * 512, (h + 1) * 512)
                    diag_here = (c < 4) == (h == 0)
                    if hilo:
                        XNh, XNl = st[b]["XNh"], st[b]["XNl"]
                        nc.tensor.matmul(
                            ps[:], XNh[:, sl], XNh[:, fs], start=True, stop=False
                        )
                        nc.tensor.matmul(
                            ps[:], XNh[:, sl], XNl[:, fs], start=False, stop=False
                        )
                        nc.tensor.matmul(
                            ps[:], XNl[:, sl], XNh[:, fs],
                            start=False, stop=not diag_here,
                        )
                    else:
                        XN = st[b]["XN"]
                        nc.tensor.matmul(
                            ps[:], XN[:, sl], XN[:, fs],
                            start=True, stop=not diag_here,
                        )
                    if diag_here:
                        # diag block -> -1e10 (accumulate -1e10*I)
                        nc.tensor.matmul(
                            ps[:, (c % 4) * 128 : (c % 4) * 128 + 128],
                            ident[:],
                            negi[:],
                            start=False,
                            stop=True,
                        )
                    nc.scalar.activation(SC[:, fs], ps[:], AF.Copy)
                if "topk" in skip:
                    nc.vector.memset(st[b]["IDX"][:, c : 64 : 8], c)
                    return
                V8 = v8p.tile([128, 8], f32, tag="v8")
                nc.vector.max(V8[:], SC[:])
                # rank-major layout: IDX[p, 8j + c] = rank-j idx of token
                # c*128 + p
                nc.vector.max_index(st[b]["IDX"][:, c : 64 : 8], V8[:], SC[:])

            def emit_C(b):
                # index shuffle into wrap-order gather layout + gathers
                # want IDXG[16g+q, 64k+8e+c] = IDX[16e+q, 8k+c]
                # DRAM layout: idxd[512q + 64k + 8e + c]; both sides are
                # 3-dim APs with 16B runs (write) / contiguous rows (read)
                IDX = st[b]["IDX"]
                idxv = idxd_h[b].rearrange("(q k e c) -> e q k c", q=16, k=8, e=8)
                for e in range(8):
                    nc.sync.dma_start(
                        out=idxv[e],
                        in_=IDX[16 * e : 16 * e + 16, :].rearrange(
                            "q (k c) -> q k c", k=8
                        ),
                    )
                IDXG = idxgp.tile([128, 512], u16, tag="idxg")
                for g in range(8):
                    nc.sync.dma_start(
                        out=IDXG[16 * g : 16 * g + 16, :],
                        in_=idxd_h[b].rearrange("(q n) -> q n", q=16),
                    )
                # gather gi covers ranks 4gi..4gi+3; position within a rank
                # block is m (wrap order), so PR[:, 1024*kl + m] = hi
                # features of the rank-(4gi+kl) neighbor of token w(m)
                prs = []
                for gi in range(2):
                    PR = prp.tile([C, 4 * N], bf16, tag="pr")
                    nc.gpsimd.dma_gather(
                        out_ap=PR[:].rearrange("p (t n) -> p t n", t=1),
                        in_ap=xt_h[b],
                        idxs_ap=IDXG[:, gi * 256 : (gi + 1) * 256].bitcast(i16),
                        num_idxs=4 * N,
                        num_idxs_reg=4 * N,
                        elem_size=C,
                        transpose=True,
                        # single_packet=True overflows the SWDGE packet limit
                        # in transpose mode and crashes the device
                        single_packet=False,
                    )
                    prs.append(PR)
                HIW = hlp.tile([C, N], bf16, tag="hiw")
                nc.sync.dma_start(out=HIW[:], in_=hiw_h[b])
                st[b]["prs"] = prs
                st[b]["HIW"] = HIW

            def emit_conv_half(b, h):
                # conv contraction (bf16 hi-only), one 512-column half
                prs, HIW = st[b]["prs"], st[b]["HIW"]
                if h == 0:
                    OUT = outp.tile([O, N], f32, tag="out")
                    st[b]["OUT"] = OUT
                OUT = st[b]["OUT"]
                PO = pso.tile([O, 512], f32, tag="ps_out")
                for k in range(1 if "conv" in skip else 9):
                    w_k = wts[:, k * O : (k + 1) * O]
                    if k == 0:
                        src = HIW[:, h * 512 : (h + 1) * 512]
                    else:
                        kl = (k - 1) % 4
                        src = prs[(k - 1) // 4][
                            :, kl * N + h * 512 : kl * N + (h + 1) * 512
                        ]
                    nc.tensor.matmul(
                        PO[:], w_k, src,
                        start=(k == 0), stop=(k == 8 or "conv" in skip),
                    )
                nc.scalar.activation(OUT[:, h * 512 : (h + 1) * 512], PO[:], AF.Copy)
                if h == 1:
                    # store from the Act queue so it never head-of-line
                    # blocks the SP queue's next-batch loads
                    nc.scalar.dma_start(out=out_h[b], in_=OUT[:])
                    del st[b]

            # -------- software-pipelined emission --------
            emit_A1(0)
            emit_A2a(0)
            emit_A2b(0)
            for b in range(BPC):
                if b >= 1:
                    emit_C(b - 1)
                IDX = idxp.tile([128, 64], u16, tag="idx")
                st[b]["IDX"] = IDX
                for c in range(8):
                    emit_sim_block(b, c)
                    if c == 0 and b + 1 < BPC:
                        emit_A1(b + 1)
                    elif c == 1 and b >= 2:
                        emit_conv_half(b - 2, 0)
                    elif c == 2 and b + 1 < BPC:
                        emit_A2a(b + 1)
                    elif c == 3 and b >= 2:
                        emit_conv_half(b - 2, 1)
                    elif c == 4 and b + 1 < BPC:
                        emit_A2b(b + 1)
            emit_C(BPC - 1)
            emit_conv_half(BPC - 2, 0)
            emit_conv_half(BPC - 2, 1)
            emit_conv_half(BPC - 1, 0)
            emit_conv_half(BPC - 1, 1)

    nc.compile()
    return nc


def _get_program():
    if "nc" not in _prog_cache:
        _prog_cache["nc"] = _build_program()
    return _prog_cache["nc"]


def _host_prep(x, W):
    """Build per-core input maps from full inputs."""
    import ml_dtypes

    bf16 = ml_dtypes.bfloat16
    xf = np.ascontiguousarray(x.reshape(B, C, N).astype(np.float32, copy=False))
    hi = xf.astype(bf16)
    wp = _wrap_perm()

    # token-major hi rows, 256B per token (natural token order)
    xt = np.ascontiguousarray(hi.transpose(0, 2, 1))
    # self features in wrap order for the k=0 conv term
    hiw = np.ascontiguousarray(hi[:, :, wp])

    wt = np.ascontiguousarray(
        np.transpose(W.astype(np.float32, copy=False), (1, 2, 0))
    ).reshape(C, K * O).astype(bf16)

    ident = np.eye(128, dtype=bf16)
    negi = (NEG * np.eye(128, dtype=np.float32)).astype(bf16)
    ones128 = np.ones((C, 1), dtype=np.float32)

    in_maps = []
    for i in range(NCORES):
        sl = slice(i * BPC, (i + 1) * BPC)
        in_maps.append(
            {
                "x": np.ascontiguousarray(xf[sl]),
                "xt": np.ascontiguousarray(xt[sl]),
                "hiw": np.ascontiguousarray(hiw[sl]),
                "wt": wt,
                "ident": ident,
                "negi": negi,
                "ones128": ones128,
            }
        )
    return in_maps


def kernel(x, W):
    global last_results
    from concourse.bass_utils import run_bass_kernel_spmd

    x = np.asarray(x)
    W = np.asarray(W)
    in_maps = _host_prep(x, W)
    nc = _get_program()
    trace = bool(int(os.environ.get("KNN_TRACE", "0")))
    res = run_bass_kernel_spmd(nc, in_maps, list(range(NCORES)), trace=trace)
    last_results = res
    wp = _wrap_perm()
    out = np.concatenate([res.results[i]["out"] for i in range(NCORES)], axis=0)
    out = out[:, :, wp]  # undo wrap order (w is an involution)
    return out.reshape(B, O, 32, 32).astype(np.float32, copy=False)


# revision 16
# speedup vs baseline: 1.1105x; 1.1105x over previous
"""KNN-conv kernel for Trainium2, data-parallel over batch on 8 NeuronCores.

Problem: for x (32, 128, 32, 32) and conv weight W (128, 128, 9):
  per batch: cosine-sim (1024x1024) over channels, diag -> -1e10, top-8
  neighbors per token (+ self as rank 0), gather neighbor features,
  contract with W.

Strategy per core (4 batches), software-pipelined at sim-block
granularity: batch b's similarity/top-k blocks carry interleaved work
from neighboring batches (conv halves of b-2, normalization of b+1) so
every engine queue stays busy and the PE stays ramped:
  A: load x, xn = x * 1/sqrt(sum x^2) (Act square/sqrt, DVE recip, Pool
     partition_broadcast + multiply, norms bounced through DRAM to get
     token-major layout)
  B: sim in true-fp32 PE matmuls (top-k selection needs ~17 mantissa
     bits; bf16/tf32-class dtypes flip ~1-3% of picks = rel err >> 2e-2),
     diag forced to -1e10 by an accumulating identity matmul, top-8 via
     DVE max8 + max_index per 128-row block
  C: index shuffle into the dma_gather wrap layout. Gather position m
     holds token w(m), where w swaps the block/slot bit-fields
     (m = 128e+16c+q -> t = 128c+16e+q), which makes the shuffle DMAs
     16B-run / contiguous instead of 2B-element scatters. Two
     dma_gather(transpose=True) calls fetch 4 neighbor ranks each from
     token-major bf16 rows (256B each).
  D: conv = single-pass bf16 matmuls (hi-only features, ~0.3% extra
     error, no top-k amplification) accumulated in fp32 PSUM; output
     columns are in wrap order, host unpermutes (w is an involution).
"""

import os

import numpy as np

B, C, N, K = 32, 128, 1024, 9
O = 128  # out channels
NCORES = 8
BPC = B // NCORES  # batches per core
NEG = -1.0e10

_prog_cache = {}
last_results = None  # BassKernelResults of the most recent run (for test.py)


def _wrap_perm():
    """w(m): swap the high (e) and mid (c) 3-bit fields of m = 128e+16c+q."""
    m = np.arange(N)
    e, c, q = m >> 7, (m >> 4) & 7, m & 15
    return (c << 7) | (e << 4) | q


def _build_program():
    import concourse.bacc as bacc
    import concourse.mybir as mybir
    from concourse.tile import TileContext

    f32 = mybir.dt.float32
    bf16 = mybir.dt.bfloat16
    u16 = mybir.dt.uint16
    i16 = mybir.dt.int16
    AF = mybir.ActivationFunctionType

    nc = bacc.Bacc()
    skip = set(os.environ.get("KNN_SKIP", "").split(","))
    hilo = os.environ.get("KNN_SIM", "f32") == "hilo"

    x_h = nc.declare_dram_parameter("x", [BPC, C, N], f32, isOutput=False)
    xt_h = nc.declare_dram_parameter("xt", [BPC, N, C], bf16, isOutput=False)
    hiw_h = nc.declare_dram_parameter("hiw", [BPC, C, N], bf16, isOutput=False)
    wt_h = nc.declare_dram_parameter("wt", [C, K * O], bf16, isOutput=False)
    ident_h = nc.declare_dram_parameter("ident", [128, 128], bf16, isOutput=False)
    negi_h = nc.declare_dram_parameter("negi", [128, 128], bf16, isOutput=False)
    ones128_h = nc.declare_dram_parameter("ones128", [C, 1], f32, isOutput=False)
    out_h = nc.declare_dram_parameter("out", [BPC, O, N], f32, isOutput=True)

    idxd_h = nc.dram_tensor("idxd", [BPC, 8192], u16)
    rd_h = nc.dram_tensor("rd", [BPC, N], f32)

    with TileContext(nc) as tc:
        with (
            tc.tile_pool(name="consts", bufs=1) as consts,
            tc.tile_pool(name="xp", bufs=3) as xp,
            tc.tile_pool(name="sqp", bufs=2) as sqp,
            tc.tile_pool(name="xnp", bufs=2) as xnp,
            tc.tile_pool(name="xnhp", bufs=2) as xnhp,
            tc.tile_pool(name="xnlp", bufs=2) as xnlp,
            tc.tile_pool(name="rp", bufs=2) as rp,
            tc.tile_pool(name="normp", bufs=6) as normp,
            tc.tile_pool(name="scp", bufs=int(os.environ.get("KNN_SCP", "3"))) as scp,
            tc.tile_pool(name="v8p", bufs=3) as v8p,
            tc.tile_pool(name="idxp", bufs=2) as idxp,
            tc.tile_pool(name="idxgp", bufs=2) as idxgp,
            tc.tile_pool(name="prp", bufs=int(os.environ.get("KNN_PRP", "6"))) as prp,
            tc.tile_pool(name="hlp", bufs=3) as hlp,
            tc.tile_pool(name="outp", bufs=2) as outp,
            tc.tile_pool(name="psb", bufs=int(os.environ.get("KNN_PSB", "6")), space="PSUM") as psb,
            tc.tile_pool(name="pso", bufs=2, space="PSUM") as pso,
        ):
            wts = consts.tile([C, K * O], bf16, tag="wts")
            nc.sync.dma_start(out=wts[:], in_=wt_h[:])
            ident = consts.tile([128, 128], bf16, tag="ident")
            nc.sync.dma_start(out=ident[:], in_=ident_h[:])
            negi = consts.tile([128, 128], bf16, tag="negi")
            nc.sync.dma_start(out=negi[:], in_=negi_h[:])
            ones128 = consts.tile([C, 1], f32, tag="ones128")
            nc.sync.dma_start(out=ones128[:], in_=ones128_h[:])

            st = {}  # per-batch live tiles

            def emit_A1_dma(b):
                # x load (dispatched first so it never queues behind the
                # previous batch's 16 shuffle DMAs)
                X = xp.tile([C, N], f32, tag="x")
                nc.sync.dma_start(out=X[:], in_=x_h[b])
                st[b] = {"X": X}

            def emit_A1_act(b):
                SQ = sqp.tile([C, N], f32, tag="sq")
                nc.scalar.activation(SQ[:], st[b]["X"][:], AF.Square)
                st[b]["SQ"] = SQ

            def emit_A2a(b):
                # norm^2 transposed: n2[p, blk] = sum_c SQ[c, blk*128+p]
                SQ = st[b]["SQ"]
                n2 = psb.tile([128, 512], f32, tag="ps_big")
                for blk in range(8):
                    nc.tensor.matmul(
                        n2[:, blk : blk + 1],
                        SQ[:, blk * 128 : (blk + 1) * 128],
                        ones128[:],
                        start=True,
                        stop=True,
                    )
                sq8 = normp.tile([128, 8], f32, tag="sq8")
                nc.scalar.activation(sq8[:], n2[:, :8], AF.Sqrt)
                st[b]["sq8"] = sq8

            def emit_A2b(b):
                # reciprocal + token-major bounce + broadcast + multiply
                X, sq8 = st[b]["X"], st[b]["sq8"]
                rT = normp.tile([128, 8], f32, tag="rT")
                nc.vector.reciprocal(rT[:], sq8[:])
                nc.sync.dma_start(
                    out=rd_h[b].rearrange("(blk p) -> p blk", p=128), in_=rT[:]
                )
                r1 = normp.tile([1, N], f32, tag="r1")
                nc.sync.dma_start(
                    out=r1[:], in_=rd_h[b].rearrange("(one n) -> one n", one=1)
                )
                R = rp.tile([128, N], f32, tag="r")
                nc.gpsimd.partition_broadcast(R[:], r1[:])
                XN = xnp.tile([C, N], f32, tag="xn")
                nc.gpsimd.tensor_mul(XN[:], X[:], R[:])
                st[b]["XN"] = XN
                if hilo:
                    XNh = xnhp.tile([C, N], bf16, tag="xnh")
                    nc.scalar.activation(XNh[:], XN[:], AF.Copy)
                    XNl = xnlp.tile([C, N], bf16, tag="xnl")
                    nc.gpsimd.tensor_sub(XNl[:], XN[:], XNh[:])
                    st[b]["XNh"], st[b]["XNl"] = XNh, XNl

            def emit_sim_block(b, c):
                # one 128-row sim block + top-8 scan
                if "sim" in skip:
                    nc.vector.memset(st[b]["IDX"][:, c : 64 : 8], c)
                    return
                sl = slice(c * 128, (c + 1) * 128)
                SC = scp.tile([128, N], f32, tag="sc")
                for h in range(2):
                    ps = psb.tile([128, 512], f32, tag="ps_big")
                    fs = slice(h * 512, (h + 1) * 512)
                    diag_here = (c < 4) == (h == 0)
                    if hilo:
                        XNh, XNl = st[b]["XNh"], st[b]["XNl"]
                        nc.tensor.matmul(
                            ps[:], XNh[:, sl], XNh[:, fs], start=True, stop=False
                        )
                        nc.tensor.matmul(
                            ps[:], XNh[:, sl], XNl[:, fs], start=False, stop=False
                        )
                        nc.tensor.matmul(
                            ps[:], XNl[:, sl], XNh[:, fs],
                            start=False, stop=not diag_here,
                        )
                    else:
                        XN = st[b]["XN"]
                        nc.tensor.matmul(
                            ps[:], XN[:, sl], XN[:, fs],
                            start=True, stop=not diag_here,
                        )
                    if diag_here:
                        # diag block -> -1e10 (accumulate -1e10*I)
                        nc.tensor.matmul(
                            ps[:, (c % 4) * 128 : (c % 4) * 128 + 128],
                            ident[:],
                            negi[:],
                            start=False,
                            stop=True,
                        )
                    nc.scalar.activation(SC[:, fs], ps[:], AF.Copy)
                if "topk" in skip:
                    nc.vector.memset(st[b]["IDX"][:, c : 64 : 8], c)
                    return
                V8 = v8p.tile([128, 8], f32, tag="v8")
                nc.vector.max(V8[:], SC[:])
                # rank-major layout: IDX[p, 8j + c] = rank-j idx of token
                # c*128 + p
                nc.vector.max_index(st[b]["IDX"][:, c : 64 : 8], V8[:], SC[:])

            def emit_C(b):
                # index shuffle into wrap-order gather layout + gathers
                # want IDXG[16g+q, 64k+8e+c] = IDX[16e+q, 8k+c]
                # DRAM layout: idxd[512q + 64k + 8e + c]; both sides are
                # 3-dim APs with 16B runs (write) / contiguous rows (read)
                IDX = st[b]["IDX"]
                idxv = idxd_h[b].rearrange("(q k e c) -> e q k c", q=16, k=8, e=8)
                for e in range(8):
                    nc.sync.dma_start(
                        out=idxv[e],
                        in_=IDX[16 * e : 16 * e + 16, :].rearrange(
                            "q (k c) -> q k c", k=8
                        ),
                    )
                IDXG = idxgp.tile([128, 512], u16, tag="idxg")
                for g in range(8):
                    nc.sync.dma_start(
                        out=IDXG[16 * g : 16 * g + 16, :],
                        in_=idxd_h[b].rearrange("(q n) -> q n", q=16),
                    )
                # gather gi covers ranks 4gi..4gi+3; position within a rank
                # block is m (wrap order), so PR[:, 1024*kl + m] = hi
                # features of the rank-(4gi+kl) neighbor of token w(m)
                prs = []
                for gi in range(2):
                    PR = prp.tile([C, 4 * N], bf16, tag="pr")
                    nc.gpsimd.dma_gather(
                        out_ap=PR[:].rearrange("p (t n) -> p t n", t=1),
                        in_ap=xt_h[b],
                        idxs_ap=IDXG[:, gi * 256 : (gi + 1) * 256].bitcast(i16),
                        num_idxs=4 * N,
                        num_idxs_reg=4 * N,
                        elem_size=C,
                        transpose=True,
                        # single_packet=True overflows the SWDGE packet limit
                        # in transpose mode and crashes the device
                        single_packet=False,
                    )
                    prs.append(PR)
                HIW = hlp.tile([C, N], bf16, tag="hiw")
                nc.sync.dma_start(out=HIW[:], in_=hiw_h[b])
                st[b]["prs"] = prs
                st[b]["HIW"] = HIW

            def emit_conv_half(b, h):
                # conv contraction (bf16 hi-only), one 512-column half
                prs, HIW = st[b]["prs"], st[b]["HIW"]
                if h == 0:
                    OUT = outp.tile([O, N], f32, tag="out")
                    st[b]["OUT"] = OUT
                OUT = st[b]["OUT"]
                PO = pso.tile([O, 512], f32, tag="ps_out")
                for k in range(1 if "conv" in skip else 9):
                    w_k = wts[:, k * O : (k + 1) * O]
                    if k == 0:
                        src = HIW[:, h * 512 : (h + 1) * 512]
                    else:
                        kl = (k - 1) % 4
                        src = prs[(k - 1) // 4][
                            :, kl * N + h * 512 : kl * N + (h + 1) * 512
                        ]
                    nc.tensor.matmul(
                        PO[:], w_k, src,
                        start=(k == 0), stop=(k == 8 or "conv" in skip),
                    )
                nc.scalar.activation(OUT[:, h * 512 : (h + 1) * 512], PO[:], AF.Copy)
                if h == 1:
                    # store from the Pool queue: SP would head-of-line block
                    # next-batch loads, Act would delay the SC copies that
                    # pace the PE/DVE pipeline
                    nc.gpsimd.dma_start(out=out_h[b], in_=OUT[:])
                    del st[b]

            # -------- software-pipelined emission --------
            emit_A1_dma(0)
            emit_A1_act(0)
            emit_A2a(0)
            emit_A2b(0)
            for b in range(BPC):
                if b + 1 < BPC:
                    emit_A1_dma(b + 1)
                if b >= 1:
                    emit_C(b - 1)
                IDX = idxp.tile([128, 64], u16, tag="idx")
                st[b]["IDX"] = IDX
                for c in range(8):
                    emit_sim_block(b, c)
                    if c == 0 and b + 1 < BPC:
                        emit_A1_act(b + 1)
                    elif c == 1 and b >= 2:
                        emit_conv_half(b - 2, 0)
                    elif c == 2 and b + 1 < BPC:
                        emit_A2a(b + 1)
                    elif c == 3 and b + 1 < BPC:
                        emit_A2b(b + 1)
                    elif c == 4 and b >= 2:
                        emit_conv_half(b - 2, 1)
            emit_C(BPC - 1)
            emit_conv_half(BPC - 2, 0)
            emit_conv_half(BPC - 2, 1)
            emit_conv_half(BPC - 1, 0)
            emit_conv_half(BPC - 1, 1)

    nc.compile()
    return nc


def _get_program():
    if "nc" not in _prog_cache:
        _prog_cache["nc"] = _build_program()
    return _prog_cache["nc"]


def _host_prep(x, W):
    """Build per-core input maps from full inputs."""
    import ml_dtypes

    bf16 = ml_dtypes.bfloat16
    xf = np.ascontiguousarray(x.reshape(B, C, N).astype(np.float32, copy=False))
    hi = xf.astype(bf16)
    wp = _wrap_perm()

    # token-major hi rows, 256B per token (natural token order)
    xt = np.ascontiguousarray(hi.transpose(0, 2, 1))
    # self features in wrap order for the k=0 conv term
    hiw = np.ascontiguousarray(hi[:, :, wp])

    wt = np.ascontiguousarray(
        np.transpose(W.astype(np.float32, copy=False), (1, 2, 0))
    ).reshape(C, K * O).astype(bf16)

    ident = np.eye(128, dtype=bf16)
    negi = (NEG * np.eye(128, dtype=np.float32)).astype(bf16)
    ones128 = np.ones((C, 1), dtype=np.float32)

    in_maps = []
    for i in range(NCORES):
        sl = slice(i * BPC, (i + 1) * BPC)
        in_maps.append(
            {
                "x": np.ascontiguousarray(xf[sl]),
                "xt": np.ascontiguousarray(xt[sl]),
                "hiw": np.ascontiguousarray(hiw[sl]),
                "wt": wt,
                "ident": ident,
                "negi": negi,
                "ones128": ones128,
            }
        )
    return in_maps


def kernel(x, W):
    global last_results
    from concourse.bass_utils import run_bass_kernel_spmd

    x = np.asarray(x)
    W = np.asarray(W)
    in_maps = _host_prep(x, W)
    nc = _get_program()
    trace = bool(int(os.environ.get("KNN_TRACE", "0")))
    res = run_bass_kernel_spmd(nc, in_maps, list(range(NCORES)), trace=trace)
    last_results = res
    wp = _wrap_perm()
    out = np.concatenate([res.results[i]["out"] for i in range(NCORES)], axis=0)
    out = out[:, :, wp]  # undo wrap order (w is an involution)
    return out.reshape(B, O, 32, 32).astype(np.float32, copy=False)
